# revision 12
# baseline (speedup 1.0000x reference)
"""Causal depthwise conv1d (K=4) + SiLU, sharded over 8 NeuronCores.

Full shapes: x [4, 8192, 2048] f32, weight [2048, 4] f32 -> y [4, 8192, 2048] f32.

Strategy: tensor-parallel over the hidden/channel dim (fully channel
independent, no halo exchange). Each core gets 256 channels, reorganized
host-side to channel-major [B*256, 3+S] (3 leading zero columns provide the
causal padding) so the conv runs along the free dim with channels on SBUF
partitions.

The kernel is HBM-bandwidth bound, so all HBM traffic is bf16: x is converted
host-side (RNE via ml_dtypes), y is written bf16 and upconverted host-side.
That halves traffic vs fp32 (rel-err budget 2e-2 >> bf16's ~2e-3).

Compute split so every engine stays under the ~2.9us/tile bf16 DMA budget:
taps 0-2 run on the TensorEngine as bf16 diagonal-matrix matmuls accumulating
in PSUM (the 32 [128,128] diag matrices are built host-side and DMA'd once);
tap 3 is folded by the DVE scalar_tensor_tensor (z = x3*w3 + psum -> sbuf
bf16); ACT applies SiLU (z -> bf16 out) and triggers the output DMA on its
own HWDGE ring, with inputs streaming on SP's ring.

Ramp shaping (the steady state already runs at the HBM roofline): the diag
DMA is split so unit 0's block lands first and tile 0's input lands in two
halves, pulling the first matmul ~5us earlier; PE/DVE work half-tiles
(per-half sem incs) and the last tile's SiLU + store are split in half, so
the end-of-kernel dependency chain drains ~6us faster.

ACT (silu + store trigger + sem waits) was measured as the steady-state
pacer at ~2.9us/tile, just above the 2.86us DMA floor, and it accumulated a
2-tile backlog by kernel end. So tiles are silued and stored in PAIRS
(adjacent tiles are contiguous in y_d): one 4096-wide ACTIVATE (saves the
352-cycle per-op ramp) and one 1MiB store trigger per two tiles puts ACT at
~2.3us/tile. The last pair keeps the v3 tail exemption (tile 30 single,
tile 31 in halves).

Raw bass (no Tile framework): the installed walrus codegen only accepts one
sync wait per compute instruction, so all synchronization is explicit wait_ge
sequencer instructions. Per-buffer-slot DMA semaphores keep concurrent DMA
completion increments unambiguous. Sem increments fire at instruction
completion, but the sequencer runs ahead, so consumers of an engine's result
always gate on that completion increment (including same-engine self-waits
before DMA triggers).
"""

import contextlib

import numpy as np

B, S, H, K = 4, 8192, 2048, 4
N_CORES = 8
HC = H // N_CORES          # 256 channels per core
ROWS = B * HC              # 1024 rows per core, row r = b*HC + c
NU = ROWS // 128           # 8 partition units
T = 2048                   # token tile
NT = S // T
NTILES = NU * NT           # 32
NB = 6                     # buffers per tile kind
NZ = 3                     # z (pre-silu) buffers
NC_CHUNK = 512             # one PSUM bank of fp32
NCHUNKS = T // NC_CHUNK
PE_TAPS = 3                # taps on the TensorEngine; tap 3 folds in on DVE
HT = T // 2                # half-tile (sem granularity for PE/DVE)
X0SPLIT = HT + 4           # tile-0 first-half DMA columns (covers chunks 0-1)

_last_results = None       # test harness introspection (exec_time_ns etc.)
_ACT_FUNC = "Silu"         # sim override hook (CoreSim lacks Silu)


def _build_program():
    from concourse import bass, mybir

    f32 = mybir.dt.float32
    bf16 = mybir.dt.bfloat16
    AF = mybir.ActivationFunctionType
    ALU = mybir.AluOpType

    nc = bass.Bass()
    # x arrives with 3 leading zero columns (causal padding): [ROWS, 3+S]
    x_d = nc.declare_dram_parameter("x", [ROWS, S + 3], bf16, isOutput=False)
    # 32 host-built [128,128] diag(w) blocks, unit-major then tap
    wd_d = nc.declare_dram_parameter("wd", [128, NU * K * 128], bf16,
                                     isOutput=False)
    # raw weights (fp32): stt scalar columns + a zeros column (Silu bias)
    w_d = nc.declare_dram_parameter("w", [128, NU * K + 1], f32, isOutput=False)
    y_d = nc.declare_dram_parameter("y", [ROWS, S], bf16, isOutput=True)

    with contextlib.ExitStack() as st:
        wt = st.enter_context(nc.sbuf_tensor("wt", [128, NU * K + 1], f32))
        wdg = st.enter_context(nc.sbuf_tensor("wdg", [128, NU * K * 128], bf16))
        xts = [
            st.enter_context(nc.sbuf_tensor(f"xt{i}", [128, T + 3], bf16))
            for i in range(NB)
        ]
        # pair-sized (2 tiles wide) pre-silu and post-silu buffers
        zps = [
            st.enter_context(nc.sbuf_tensor(f"zp{i}", [128, 2 * T], bf16))
            for i in range(NZ)
        ]
        ytp = [
            st.enter_context(nc.sbuf_tensor(f"yp{i}", [128, 2 * T], bf16))
            for i in range(NZ)
        ]
        pss = [
            st.enter_context(nc.psum_tensor(f"ps{i}", [128, T], f32))
            for i in range(2)
        ]
        zb = wt[:, NU * K : NU * K + 1]           # zeros column (Silu bias)

        def wdiag(k, i):
            u = k // NT
            c0 = (u * K + i) * 128
            return wdg[:, c0 : c0 + 128]

        def w3col(k):
            u = k // NT
            return wt[:, u * K + 3 : u * K + 4]

        def x_rows(k):
            r0 = (k // NT) * 128
            return r0, r0 + 128

        # cumulative din counts: tile 0 arrives as two half DMAs
        din_need = []          # din count PE needs before tile k (full tile)
        din_tot = [0] * NB
        for k in range(NTILES):
            din_tot[k % NB] += 32 if k == 0 else 16
            din_need.append(din_tot[k % NB])

        with (
            nc.Block() as block,
            nc.semaphore("wsem") as wsem,
            nc.semaphore("dsem0") as dsem0,
            nc.semaphore("dsem1") as dsem1,
            nc.semaphore("dvez") as dvez,
            nc.semaphore("act") as act,
            nc.semaphore("pe") as pe,
            contextlib.ExitStack() as sems,
        ):
            din = [
                sems.enter_context(nc.semaphore(f"din{i}")) for i in range(NB)
            ]
            dout = [
                sems.enter_context(nc.semaphore(f"dout{i}")) for i in range(NZ)
            ]

            @block.sync
            def _(sync):
                for k in range(NTILES):
                    r0, r1 = x_rows(k)
                    t0 = (k % NT) * T
                    if k >= NB:
                        # xt slot free once DVE folded tap 3 of tile k-NB
                        sync.wait_ge(dvez, k - NB + 1)
                    # padded coords: window [t0-3, t0+T) = x_d cols [t0, t0+T+3)
                    if k == 0:
                        sync.dma_start(
                            out=xts[0][:, :X0SPLIT],
                            in_=x_d[r0:r1, :X0SPLIT],
                        ).then_inc(din[0], 16)
                        sync.dma_start(
                            out=xts[0][:, X0SPLIT : T + 3],
                            in_=x_d[r0:r1, X0SPLIT : T + 3],
                        ).then_inc(din[0], 16)
                    else:
                        sync.dma_start(
                            out=xts[k % NB][:, :],
                            in_=x_d[r0:r1, t0 : t0 + T + 3],
                        ).then_inc(din[k % NB], 16)

            @block.tensor
            def _(tensor):
                tensor.wait_ge(dsem0, 16)
                for k in range(NTILES):
                    if k == 4:
                        tensor.wait_ge(dsem1, 16)
                    if k >= 2:
                        # psum buffer free once DVE consumed tile k-2
                        tensor.wait_ge(dvez, k - 1)
                    ps = pss[k % 2]
                    xt = xts[k % NB]
                    for h in range(2):
                        if k == 0:
                            tensor.wait_ge(din[0], 16 * (h + 1))
                        elif h == 0:
                            tensor.wait_ge(din[k % NB], din_need[k])
                        for c in range(2 * h, 2 * h + 2):
                            c0 = c * NC_CHUNK
                            for i in range(PE_TAPS):
                                mm = tensor.matmul(
                                    ps[:, c0 : c0 + NC_CHUNK],
                                    wdiag(k, i),
                                    xt[:, c0 + i : c0 + i + NC_CHUNK],
                                    start=(i == 0),
                                    stop=(i == PE_TAPS - 1),
                                    skip_group_check=True,
                                )
                        mm.then_inc(pe)

            @block.vector
            def _(vector):
                vector.wait_ge(wsem, 16)
                for k in range(NTILES):
                    j, e = k // 2, k % 2
                    if e == 0 and j >= NZ:
                        # z pair slot free once ACT silued pair j-NZ
                        vector.wait_ge(act, j - NZ + 1)
                    # full-tile stt in steady state (DVE-cheapest); halves for
                    # the final two tiles so the tail drains at finer grain
                    halves = (0, 1) if k >= NTILES - 2 else (None,)
                    for h in halves:
                        h0 = 0 if h is None else h * HT
                        w = T if h is None else HT
                        vector.wait_ge(pe, 2 * k + 2 if h is None
                                       else 2 * k + h + 1)
                        # z = x3 * w3 + psum  (tap 3 fold)
                        vector.scalar_tensor_tensor(
                            out=zps[j % NZ][:, e * T + h0 : e * T + h0 + w],
                            in0=xts[k % NB][:, 3 + h0 : 3 + h0 + w],
                            scalar=w3col(k),
                            in1=pss[k % 2][:, h0 : h0 + w],
                            op0=ALU.mult,
                            op1=ALU.add,
                        ).then_inc(dvez)

            @block.scalar
            def _(scalar):
                scalar.dma_start(
                    out=wdg[:, : K * 128], in_=wd_d[:, : K * 128]
                ).then_inc(dsem0, 16)
                scalar.dma_start(out=wt[:, :], in_=w_d[:, :]).then_inc(wsem, 16)
                scalar.dma_start(
                    out=wdg[:, K * 128 :], in_=wd_d[:, K * 128 :]
                ).then_inc(dsem1, 16)
                func = getattr(AF, _ACT_FUNC)
                bias = 0.0 if func == AF.Copy else zb
                NPAIR = NTILES // 2
                n_act = 0
                n_store = [0] * NZ

                def silu_store(j, sl, dvez_need):
                    # silu zps[j%NZ][:, sl] -> ytp, then store that slice
                    nonlocal n_act
                    k0 = 2 * j
                    r0, r1 = x_rows(k0)
                    t0 = (k0 % NT) * T
                    scalar.wait_ge(dvez, dvez_need)
                    scalar.activation(
                        out=ytp[j % NZ][:, sl], in_=zps[j % NZ][:, sl],
                        func=func, bias=bias, scale=1.0,
                    ).then_inc(act)
                    n_act += 1
                    # the DMA trigger races ahead of the still-streaming
                    # activation write; self-wait on its completion inc
                    scalar.wait_ge(act, n_act)
                    scalar.dma_start(
                        out=y_d[r0:r1, t0 + sl.start : t0 + sl.stop],
                        in_=ytp[j % NZ][:, sl],
                    ).then_inc(dout[j % NZ], 16)
                    n_store[j % NZ] += 1

                for j in range(NPAIR):
                    if j >= NZ:
                        # yt pair slot's previous store must be done
                        scalar.wait_ge(dout[j % NZ], 16 * (j // NZ))
                    if j < NPAIR - 1:
                        silu_store(j, slice(0, 2 * T), 2 * j + 2)
                    else:
                        # last pair: half grain to shorten the drain chain
                        # (dvez counts: tiles 0-29 one inc, 30/31 two each)
                        silu_store(j, slice(0, HT), 31)
                        silu_store(j, slice(HT, T), 32)
                        silu_store(j, slice(T, T + HT), 33)
                        silu_store(j, slice(T + HT, 2 * T), 34)
                for i in range(NZ):
                    scalar.wait_ge(dout[i], 16 * n_store[i])

    return nc


def kernel(x, weight):
    global _last_results
    import ml_dtypes
    from concourse.bass_utils import run_bass_kernel_spmd

    bf16 = ml_dtypes.bfloat16
    x = np.asarray(x, dtype=np.float32)
    weight = np.asarray(weight, dtype=np.float32)

    nc = _build_program()

    in_maps = []
    for core in range(N_CORES):
        sl = slice(core * HC, (core + 1) * HC)
        # [B, S, HC] -> [B, HC, S] -> [ROWS, S] with 3 leading zero columns
        # (the causal padding), row r = b*HC + c
        xs = np.zeros((ROWS, S + 3), bf16)
        xs[:, 3:] = x[:, :, sl].transpose(0, 2, 1).reshape(ROWS, S).astype(bf16)
        ws = weight[sl, :]  # (HC, K)
        w_host = np.zeros((128, NU * K + 1), np.float32)
        wd_host = np.zeros((128, NU * K * 128), bf16)
        idx = np.arange(128)
        for u in range(NU):
            blk = u % (HC // 128)
            wu = ws[blk * 128 : (blk + 1) * 128, :]  # (128, K)
            w_host[:, u * K : (u + 1) * K] = wu
            for i in range(K):
                wd_host[idx, (u * K + i) * 128 + idx] = wu[:, i].astype(bf16)
        in_maps.append({"x": xs, "w": w_host, "wd": wd_host})

    res = run_bass_kernel_spmd(nc, in_maps, list(range(N_CORES)))
    _last_results = res

    out = np.empty((B, S, H), np.float32)
    for core in range(N_CORES):
        sl = slice(core * HC, (core + 1) * HC)
        yc = res.results[core]["y"].astype(np.float32).reshape(B, HC, S)
        out[:, :, sl] = yc.transpose(0, 2, 1)
    return out


# revision 19
# speedup vs baseline: 1.0162x; 1.0162x over previous
"""Causal depthwise conv1d (K=4) + SiLU, sharded over 8 NeuronCores.

Full shapes: x [4, 8192, 2048] f32, weight [2048, 4] f32 -> y [4, 8192, 2048] f32.

Strategy: tensor-parallel over the hidden/channel dim (fully channel
independent, no halo exchange). Each core gets 256 channels, reorganized
host-side to channel-major [B*256, 3+S] (3 leading zero columns provide the
causal padding) so the conv runs along the free dim with channels on SBUF
partitions.

The kernel is HBM-bandwidth bound, so all HBM traffic is bf16: x is converted
host-side (RNE via ml_dtypes), y is written bf16 and upconverted host-side.
That halves traffic vs fp32 (rel-err budget 2e-2 >> bf16's ~2e-3).

Compute split so every engine stays under the ~2.9us/tile bf16 DMA budget:
taps 0-2 run on the TensorEngine as bf16 diagonal-matrix matmuls accumulating
in PSUM (the 32 [128,128] diag matrices are built host-side and DMA'd once);
tap 3 is folded by the DVE scalar_tensor_tensor (z = x3*w3 + psum -> sbuf
bf16); ACT applies SiLU (z -> bf16 out) and triggers the output DMA on its
own HWDGE ring, with inputs streaming on SP's ring.

Ramp shaping (the steady state already runs at the HBM roofline): the diag
DMA is split so unit 0's block lands first and tile 0's input lands in two
halves, pulling the first matmul ~5us earlier; PE/DVE work half-tiles
(per-half sem incs) and the last tile's SiLU + store are split in half, so
the end-of-kernel dependency chain drains ~6us faster.

ACT (silu + store trigger + sem waits) was measured as the steady-state
pacer at ~2.9us/tile, just above the 2.86us DMA floor, and it accumulated a
2-tile backlog by kernel end. So tiles are silued and stored in PAIRS
(adjacent tiles are contiguous in y_d): one 4096-wide ACTIVATE (saves the
352-cycle per-op ramp) and one 1MiB store trigger per two tiles puts ACT at
~2.3us/tile. The last pair keeps the v3 tail exemption (tile 30 single,
tile 31 in halves).

Raw bass (no Tile framework): the installed walrus codegen only accepts one
sync wait per compute instruction, so all synchronization is explicit wait_ge
sequencer instructions. Per-buffer-slot DMA semaphores keep concurrent DMA
completion increments unambiguous. Sem increments fire at instruction
completion, but the sequencer runs ahead, so consumers of an engine's result
always gate on that completion increment (including same-engine self-waits
before DMA triggers).
"""

import contextlib

import numpy as np

B, S, H, K = 4, 8192, 2048, 4
N_CORES = 8
HC = H // N_CORES          # 256 channels per core
ROWS = B * HC              # 1024 rows per core, row r = b*HC + c
NU = ROWS // 128           # 8 partition units
T = 2048                   # token tile
NT = S // T
NTILES = NU * NT           # 32
NB = 10                    # x-tile buffers (deep prefetch rides HBM jitter)
NZ = 3                     # z (pre-silu) buffers
NWARM = 9                  # dummy matmuls to lift the PE HAM clock-gate
NC_CHUNK = 512             # one PSUM bank of fp32
NCHUNKS = T // NC_CHUNK
PE_TAPS = 3                # taps on the TensorEngine; tap 3 folds in on DVE
HT = T // 2                # half-tile (sem granularity for PE/DVE)
X0SPLIT = HT + 4           # tile-0 first-half DMA columns (covers chunks 0-1)

_last_results = None       # test harness introspection (exec_time_ns etc.)
_ACT_FUNC = "Silu"         # sim override hook (CoreSim lacks Silu)


def _build_program():
    from concourse import bass, mybir

    f32 = mybir.dt.float32
    bf16 = mybir.dt.bfloat16
    AF = mybir.ActivationFunctionType
    ALU = mybir.AluOpType

    nc = bass.Bass()
    # x arrives with 3 leading zero columns (causal padding): [ROWS, 3+S]
    x_d = nc.declare_dram_parameter("x", [ROWS, S + 3], bf16, isOutput=False)
    # 32 host-built [128,128] diag(w) blocks, unit-major then tap
    wd_d = nc.declare_dram_parameter("wd", [128, NU * K * 128], bf16,
                                     isOutput=False)
    # raw weights (fp32): stt scalar columns + a zeros column (Silu bias)
    w_d = nc.declare_dram_parameter("w", [128, NU * K + 1], f32, isOutput=False)
    y_d = nc.declare_dram_parameter("y", [ROWS, S], bf16, isOutput=True)

    with contextlib.ExitStack() as st:
        wt = st.enter_context(nc.sbuf_tensor("wt", [128, NU * K + 1], f32))
        wdg = st.enter_context(nc.sbuf_tensor("wdg", [128, NU * K * 128], bf16))
        xts = [
            st.enter_context(nc.sbuf_tensor(f"xt{i}", [128, T + 3], bf16))
            for i in range(NB)
        ]
        # pair-sized (2 tiles wide) pre-silu and post-silu buffers
        zps = [
            st.enter_context(nc.sbuf_tensor(f"zp{i}", [128, 2 * T], bf16))
            for i in range(NZ)
        ]
        ytp = [
            st.enter_context(nc.sbuf_tensor(f"yp{i}", [128, 2 * T], bf16))
            for i in range(NZ)
        ]
        pss = [
            st.enter_context(nc.psum_tensor(f"ps{i}", [128, T], f32))
            for i in range(2)
        ]
        scr = st.enter_context(nc.sbuf_tensor("scr", [128, NC_CHUNK], bf16))
        zb = wt[:, NU * K : NU * K + 1]           # zeros column (Silu bias)

        def wdiag(k, i):
            u = k // NT
            c0 = (u * K + i) * 128
            return wdg[:, c0 : c0 + 128]

        def w3col(k):
            u = k // NT
            return wt[:, u * K + 3 : u * K + 4]

        def x_rows(k):
            r0 = (k // NT) * 128
            return r0, r0 + 128

        # cumulative din counts: tile 0 arrives as two half DMAs
        din_need = []          # din count PE needs before tile k (full tile)
        din_tot = [0] * NB
        for k in range(NTILES):
            din_tot[k % NB] += 32 if k == 0 else 16
            din_need.append(din_tot[k % NB])

        with (
            nc.Block() as block,
            nc.semaphore("wsem") as wsem,
            nc.semaphore("dsem0") as dsem0,
            nc.semaphore("dsem1") as dsem1,
            nc.semaphore("dvez") as dvez,
            nc.semaphore("act") as act,
            nc.semaphore("pe") as pe,
            nc.semaphore("warm") as warm,
            contextlib.ExitStack() as sems,
        ):
            din = [
                sems.enter_context(nc.semaphore(f"din{i}")) for i in range(NB)
            ]
            dout = [
                sems.enter_context(nc.semaphore(f"dout{i}")) for i in range(NZ)
            ]

            @block.sync
            def _(sync):
                for k in range(NTILES):
                    r0, r1 = x_rows(k)
                    t0 = (k % NT) * T
                    if k >= NB:
                        # xt slot free once DVE folded tap 3 of tile k-NB
                        sync.wait_ge(dvez, k - NB + 1)
                    # padded coords: window [t0-3, t0+T) = x_d cols [t0, t0+T+3)
                    if k == 0:
                        sync.dma_start(
                            out=xts[0][:, :X0SPLIT],
                            in_=x_d[r0:r1, :X0SPLIT],
                        ).then_inc(din[0], 16)
                        sync.dma_start(
                            out=xts[0][:, X0SPLIT : T + 3],
                            in_=x_d[r0:r1, X0SPLIT : T + 3],
                        ).then_inc(din[0], 16)
                    else:
                        sync.dma_start(
                            out=xts[k % NB][:, :],
                            in_=x_d[r0:r1, t0 : t0 + T + 3],
                        ).then_inc(din[k % NB], 16)

            @block.tensor
            def _(tensor):
                tensor.wait_ge(dsem0, 16)
                for k in range(NTILES):
                    if k == 4:
                        tensor.wait_ge(dsem1, 16)
                    if k >= 2:
                        # psum buffer free once DVE consumed tile k-2
                        tensor.wait_ge(dvez, k - 1)
                    ps = pss[k % 2]
                    xt = xts[k % NB]
                    for h in range(2):
                        if k == 0:
                            tensor.wait_ge(din[0], 16 * (h + 1))
                        elif h == 0:
                            tensor.wait_ge(din[k % NB], din_need[k])
                        for c in range(2 * h, 2 * h + 2):
                            c0 = c * NC_CHUNK
                            for i in range(PE_TAPS):
                                mm = tensor.matmul(
                                    ps[:, c0 : c0 + NC_CHUNK],
                                    wdiag(k, i),
                                    xt[:, c0 + i : c0 + i + NC_CHUNK],
                                    start=(i == 0),
                                    stop=(i == PE_TAPS - 1),
                                    skip_group_check=True,
                                )
                        if h == 1 or k >= NTILES - 2:
                            # one inc per steady tile; halves for the last two
                            # (pe counts: k+1 for k<=29, then 31,32,33,34)
                            mm.then_inc(pe)

            @block.vector
            def _(vector):
                vector.wait_ge(wsem, 16)
                for k in range(NTILES):
                    j, e = k // 2, k % 2
                    if e == 0 and j >= NZ:
                        # z pair slot free once ACT silued pair j-NZ
                        vector.wait_ge(act, j - NZ + 1)
                    # full-tile stt in steady state (DVE-cheapest); halves for
                    # the final two tiles so the tail drains at finer grain
                    halves = (0, 1) if k >= NTILES - 2 else (None,)
                    for h in halves:
                        h0 = 0 if h is None else h * HT
                        w = T if h is None else HT
                        # pe counts: k+1 for k<=29, then halves 31,32,33,34
                        vector.wait_ge(pe, k + 1 if h is None
                                       else 2 * k + h - 29)
                        # z = x3 * w3 + psum  (tap 3 fold)
                        vector.scalar_tensor_tensor(
                            out=zps[j % NZ][:, e * T + h0 : e * T + h0 + w],
                            in0=xts[k % NB][:, 3 + h0 : 3 + h0 + w],
                            scalar=w3col(k),
                            in1=pss[k % 2][:, h0 : h0 + w],
                            op0=ALU.mult,
                            op1=ALU.add,
                        ).then_inc(dvez)

            @block.scalar
            def _(scalar):
                scalar.dma_start(
                    out=wdg[:, : K * 128], in_=wd_d[:, : K * 128]
                ).then_inc(dsem0, 16)
                scalar.dma_start(out=wt[:, :], in_=w_d[:, :]).then_inc(wsem, 16)
                scalar.dma_start(
                    out=wdg[:, K * 128 :], in_=wd_d[:, K * 128 :]
                ).then_inc(dsem1, 16)
                func = getattr(AF, _ACT_FUNC)
                bias = 0.0 if func == AF.Copy else zb
                NPAIR = NTILES // 2
                n_act = 0
                n_store = [0] * NZ

                def silu_store(j, sl, dvez_need):
                    # silu zps[j%NZ][:, sl] -> ytp, then store that slice
                    nonlocal n_act
                    k0 = 2 * j
                    r0, r1 = x_rows(k0)
                    t0 = (k0 % NT) * T
                    scalar.wait_ge(dvez, dvez_need)
                    scalar.activation(
                        out=ytp[j % NZ][:, sl], in_=zps[j % NZ][:, sl],
                        func=func, bias=bias, scale=1.0,
                    ).then_inc(act)
                    n_act += 1
                    # the DMA trigger races ahead of the still-streaming
                    # activation write; self-wait on its completion inc
                    scalar.wait_ge(act, n_act)
                    scalar.dma_start(
                        out=y_d[r0:r1, t0 + sl.start : t0 + sl.stop],
                        in_=ytp[j % NZ][:, sl],
                    ).then_inc(dout[j % NZ], 16)
                    n_store[j % NZ] += 1

                for j in range(NPAIR):
                    if j >= NZ:
                        # yt pair slot's previous store must be done
                        scalar.wait_ge(dout[j % NZ], 16 * (j // NZ))
                    if j < NPAIR - 2:
                        silu_store(j, slice(0, 2 * T), 2 * j + 2)
                    elif j == NPAIR - 2:
                        # tail: single tiles so ACT never backlogs behind a
                        # pair-sized silu while the final tiles drain
                        silu_store(j, slice(0, T), 29)
                        silu_store(j, slice(T, 2 * T), 30)
                    else:
                        # final pair: half grain
                        # (dvez counts: tiles 0-29 one inc, 30/31 two each)
                        silu_store(j, slice(0, HT), 31)
                        silu_store(j, slice(HT, T), 32)
                        silu_store(j, slice(T, T + HT), 33)
                        silu_store(j, slice(T + HT, 2 * T), 34)
                for i in range(NZ):
                    scalar.wait_ge(dout[i], 16 * n_store[i])

    return nc


def kernel(x, weight):
    global _last_results
    import ml_dtypes
    from concourse.bass_utils import run_bass_kernel_spmd

    bf16 = ml_dtypes.bfloat16
    x = np.asarray(x, dtype=np.float32)
    weight = np.asarray(weight, dtype=np.float32)

    nc = _build_program()

    in_maps = []
    for core in range(N_CORES):
        sl = slice(core * HC, (core + 1) * HC)
        # [B, S, HC] -> [B, HC, S] -> [ROWS, S] with 3 leading zero columns
        # (the causal padding), row r = b*HC + c
        xs = np.zeros((ROWS, S + 3), bf16)
        xs[:, 3:] = x[:, :, sl].transpose(0, 2, 1).reshape(ROWS, S).astype(bf16)
        ws = weight[sl, :]  # (HC, K)
        w_host = np.zeros((128, NU * K + 1), np.float32)
        wd_host = np.zeros((128, NU * K * 128), bf16)
        idx = np.arange(128)
        for u in range(NU):
            blk = u % (HC // 128)
            wu = ws[blk * 128 : (blk + 1) * 128, :]  # (128, K)
            w_host[:, u * K : (u + 1) * K] = wu
            for i in range(K):
                wd_host[idx, (u * K + i) * 128 + idx] = wu[:, i].astype(bf16)
        in_maps.append({"x": xs, "w": w_host, "wd": wd_host})

    res = run_bass_kernel_spmd(nc, in_maps, list(range(N_CORES)))
    _last_results = res

    out = np.empty((B, S, H), np.float32)
    for core in range(N_CORES):
        sl = slice(core * HC, (core + 1) * HC)
        yc = res.results[core]["y"].astype(np.float32).reshape(B, HC, S)
        out[:, :, sl] = yc.transpose(0, 2, 1)
    return out


# revision 25
# speedup vs baseline: 1.0232x; 1.0068x over previous
"""Causal depthwise conv1d (K=4) + SiLU, sharded over 8 NeuronCores.

Full shapes: x [4, 8192, 2048] f32, weight [2048, 4] f32 -> y [4, 8192, 2048] f32.

Strategy: tensor-parallel over the hidden/channel dim (fully channel
independent, no halo exchange). Each core gets 256 channels, reorganized
host-side to channel-major [B*256, 3+S] (3 leading zero columns provide the
causal padding) so the conv runs along the free dim with channels on SBUF
partitions.

The kernel is HBM-bandwidth bound, so all HBM traffic is bf16: x is converted
host-side (RNE via ml_dtypes), y is written bf16 and upconverted host-side.
That halves traffic vs fp32 (rel-err budget 2e-2 >> bf16's ~2e-3).

Compute split so every engine stays under the ~2.9us/tile bf16 DMA budget:
taps 0-2 run on the TensorEngine as bf16 diagonal-matrix matmuls accumulating
in PSUM (the 32 [128,128] diag matrices are built host-side and DMA'd once);
tap 3 is folded by the DVE scalar_tensor_tensor (z = x3*w3 + psum -> sbuf
bf16); ACT applies SiLU (z -> bf16 out) and triggers the output DMA on its
own HWDGE ring, with inputs streaming on SP's ring.

Ramp shaping (the steady state already runs at the HBM roofline): the diag
DMA is split so unit 0's block lands first and tile 0's input lands in two
halves, pulling the first matmul ~5us earlier; PE/DVE work half-tiles
(per-half sem incs) and the last tile's SiLU + store are split in half, so
the end-of-kernel dependency chain drains ~6us faster.

ACT (silu + store trigger + sem waits) was measured as the steady-state
pacer at ~2.9us/tile, just above the 2.86us DMA floor, and it accumulated a
2-tile backlog by kernel end. So tiles are silued and stored in PAIRS
(adjacent tiles are contiguous in y_d): one 4096-wide ACTIVATE (saves the
352-cycle per-op ramp) and one 1MiB store trigger per two tiles puts ACT at
~2.3us/tile. The last pair keeps the v3 tail exemption (tile 30 single,
tile 31 in halves).

Raw bass (no Tile framework): the installed walrus codegen only accepts one
sync wait per compute instruction, so all synchronization is explicit wait_ge
sequencer instructions. Per-buffer-slot DMA semaphores keep concurrent DMA
completion increments unambiguous. Sem increments fire at instruction
completion, but the sequencer runs ahead, so consumers of an engine's result
always gate on that completion increment (including same-engine self-waits
before DMA triggers).
"""

import contextlib

import numpy as np

B, S, H, K = 4, 8192, 2048, 4
N_CORES = 8
HC = H // N_CORES          # 256 channels per core
ROWS = B * HC              # 1024 rows per core, row r = b*HC + c
NU = ROWS // 128           # 8 partition units
T = 2048                   # token tile
NT = S // T
NTILES = NU * NT           # 32
NB = 10                    # x-tile buffers (deep prefetch rides HBM jitter)
NZ = 4                     # z/y pair buffers (pair-silu blocks need slack)
NC_CHUNK = 512             # one PSUM bank of fp32
NCHUNKS = T // NC_CHUNK
PE_TAPS = 3                # taps on the TensorEngine; tap 3 folds in on DVE
HT = T // 2                # half-tile (sem granularity for PE/DVE)
X0SPLIT = HT + 4           # tile-0 first-half DMA columns (covers chunks 0-1)

_last_results = None       # test harness introspection (exec_time_ns etc.)
_ACT_FUNC = "Silu"         # sim override hook (CoreSim lacks Silu)


def _build_program():
    from concourse import bass, mybir

    f32 = mybir.dt.float32
    bf16 = mybir.dt.bfloat16
    AF = mybir.ActivationFunctionType
    ALU = mybir.AluOpType

    nc = bass.Bass()
    # x arrives with 3 leading zero columns (causal padding): [ROWS, 3+S]
    x_d = nc.declare_dram_parameter("x", [ROWS, S + 3], bf16, isOutput=False)
    # 32 host-built [128,128] diag(w) blocks, unit-major then tap
    wd_d = nc.declare_dram_parameter("wd", [128, NU * K * 128], bf16,
                                     isOutput=False)
    # raw weights (fp32): stt scalar columns + a zeros column (Silu bias)
    w_d = nc.declare_dram_parameter("w", [128, NU * K + 1], f32, isOutput=False)
    y_d = nc.declare_dram_parameter("y", [ROWS, S], bf16, isOutput=True)

    with contextlib.ExitStack() as st:
        wt = st.enter_context(nc.sbuf_tensor("wt", [128, NU * K + 1], f32))
        wdg = st.enter_context(nc.sbuf_tensor("wdg", [128, NU * K * 128], bf16))
        xts = [
            st.enter_context(nc.sbuf_tensor(f"xt{i}", [128, T + 3], bf16))
            for i in range(NB)
        ]
        # pair-sized (2 tiles wide) pre-silu and post-silu buffers
        zps = [
            st.enter_context(nc.sbuf_tensor(f"zp{i}", [128, 2 * T], bf16))
            for i in range(NZ)
        ]
        ytp = [
            st.enter_context(nc.sbuf_tensor(f"yp{i}", [128, 2 * T], bf16))
            for i in range(NZ)
        ]
        pss = [
            st.enter_context(nc.psum_tensor(f"ps{i}", [128, T], f32))
            for i in range(2)
        ]
        zb = wt[:, NU * K : NU * K + 1]           # zeros column (Silu bias)

        def wdiag(k, i):
            u = k // NT
            c0 = (u * K + i) * 128
            return wdg[:, c0 : c0 + 128]

        def w3col(k):
            u = k // NT
            return wt[:, u * K + 3 : u * K + 4]

        def x_rows(k):
            r0 = (k // NT) * 128
            return r0, r0 + 128

        # cumulative din counts: tile 0 arrives as two half DMAs
        din_need = []          # din count PE needs before tile k (full tile)
        din_tot = [0] * NB
        for k in range(NTILES):
            din_tot[k % NB] += 32 if k == 0 else 16
            din_need.append(din_tot[k % NB])

        with (
            nc.Block() as block,
            nc.semaphore("wsem") as wsem,
            nc.semaphore("dsem0") as dsem0,
            nc.semaphore("dsem1") as dsem1,
            nc.semaphore("dvez") as dvez,
            nc.semaphore("act") as act,
            nc.semaphore("pe") as pe,
            contextlib.ExitStack() as sems,
        ):
            din = [
                sems.enter_context(nc.semaphore(f"din{i}")) for i in range(NB)
            ]
            dout = [
                sems.enter_context(nc.semaphore(f"dout{i}")) for i in range(NZ)
            ]

            @block.sync
            def _(sync):
                for k in range(NTILES):
                    r0, r1 = x_rows(k)
                    t0 = (k % NT) * T
                    if k >= NB:
                        # xt slot free once DVE folded tap 3 of tile k-NB
                        sync.wait_ge(dvez, k - NB + 1)
                    # padded coords: window [t0-3, t0+T) = x_d cols [t0, t0+T+3)
                    if k == 0:
                        sync.dma_start(
                            out=xts[0][:, :X0SPLIT],
                            in_=x_d[r0:r1, :X0SPLIT],
                        ).then_inc(din[0], 16)
                        sync.dma_start(
                            out=xts[0][:, X0SPLIT : T + 3],
                            in_=x_d[r0:r1, X0SPLIT : T + 3],
                        ).then_inc(din[0], 16)
                    else:
                        sync.dma_start(
                            out=xts[k % NB][:, :],
                            in_=x_d[r0:r1, t0 : t0 + T + 3],
                        ).then_inc(din[k % NB], 16)

            @block.tensor
            def _(tensor):
                tensor.wait_ge(dsem0, 16)
                for k in range(NTILES):
                    if k == 4:
                        tensor.wait_ge(dsem1, 16)
                    if k >= 2:
                        # psum buffer free once DVE consumed tile k-2
                        tensor.wait_ge(dvez, k - 1)
                    ps = pss[k % 2]
                    xt = xts[k % NB]
                    for h in range(2):
                        if k == 0:
                            tensor.wait_ge(din[0], 16 * (h + 1))
                        elif h == 0:
                            tensor.wait_ge(din[k % NB], din_need[k])
                        for c in range(2 * h, 2 * h + 2):
                            c0 = c * NC_CHUNK
                            for i in range(PE_TAPS):
                                mm = tensor.matmul(
                                    ps[:, c0 : c0 + NC_CHUNK],
                                    wdiag(k, i),
                                    xt[:, c0 + i : c0 + i + NC_CHUNK],
                                    start=(i == 0),
                                    stop=(i == PE_TAPS - 1),
                                    skip_group_check=True,
                                )
                        if h == 1 or k == NTILES - 1:
                            # one inc per steady tile; halves for the last
                            # (pe counts: k+1 for k<=30, then 32, 33)
                            mm.then_inc(pe)

            @block.vector
            def _(vector):
                vector.wait_ge(wsem, 16)
                for k in range(NTILES):
                    j, e = k // 2, k % 2
                    if e == 0 and j >= NZ:
                        # z pair slot free once ACT silued pair j-NZ
                        vector.wait_ge(act, j - NZ + 1)
                    # full-tile stt in steady state (DVE-cheapest); halves for
                    # the final tile so the tail drains at finer grain
                    halves = (0, 1) if k == NTILES - 1 else (None,)
                    for h in halves:
                        h0 = 0 if h is None else h * HT
                        w = T if h is None else HT
                        # pe counts: k+1 for k<=30, then halves 32, 33
                        vector.wait_ge(pe, k + 1 if h is None else 32 + h)
                        # z = x3 * w3 + psum  (tap 3 fold)
                        vector.scalar_tensor_tensor(
                            out=zps[j % NZ][:, e * T + h0 : e * T + h0 + w],
                            in0=xts[k % NB][:, 3 + h0 : 3 + h0 + w],
                            scalar=w3col(k),
                            in1=pss[k % 2][:, h0 : h0 + w],
                            op0=ALU.mult,
                            op1=ALU.add,
                        ).then_inc(dvez)

            @block.scalar
            def _(scalar):
                scalar.dma_start(
                    out=wdg[:, : K * 128], in_=wd_d[:, : K * 128]
                ).then_inc(dsem0, 16)
                scalar.dma_start(out=wt[:, :], in_=w_d[:, :]).then_inc(wsem, 16)
                scalar.dma_start(
                    out=wdg[:, K * 128 :], in_=wd_d[:, K * 128 :]
                ).then_inc(dsem1, 16)
                func = getattr(AF, _ACT_FUNC)
                bias = 0.0 if func == AF.Copy else zb
                NPAIR = NTILES // 2
                n_act = 0
                n_store = [0] * NZ

                def silu_store(j, sl, dvez_need):
                    # silu zps[j%NZ][:, sl] -> ytp, then store that slice
                    nonlocal n_act
                    k0 = 2 * j
                    r0, r1 = x_rows(k0)
                    t0 = (k0 % NT) * T
                    scalar.wait_ge(dvez, dvez_need)
                    scalar.activation(
                        out=ytp[j % NZ][:, sl], in_=zps[j % NZ][:, sl],
                        func=func, bias=bias, scale=1.0,
                    ).then_inc(act)
                    n_act += 1
                    # the DMA trigger races ahead of the still-streaming
                    # activation write; self-wait on its completion inc
                    scalar.wait_ge(act, n_act)
                    scalar.dma_start(
                        out=y_d[r0:r1, t0 + sl.start : t0 + sl.stop],
                        in_=ytp[j % NZ][:, sl],
                    ).then_inc(dout[j % NZ], 16)
                    n_store[j % NZ] += 1

                for j in range(NPAIR):
                    if j >= NZ:
                        # yt pair slot's previous store must be done
                        scalar.wait_ge(dout[j % NZ], 16 * (j // NZ))
                    if j < NPAIR - 2:
                        silu_store(j, slice(0, 2 * T), 2 * j + 2)
                    elif j == NPAIR - 2:
                        # tail: single tiles so ACT never backlogs behind a
                        # pair-sized silu while the final tiles drain
                        silu_store(j, slice(0, T), 29)
                        silu_store(j, slice(T, 2 * T), 30)
                    else:
                        # final pair: tile 30 single, tile 31 in halves
                        # (dvez counts: tiles 0-30 one inc, 31 two)
                        silu_store(j, slice(0, T), 31)
                        silu_store(j, slice(T, T + HT), 32)
                        silu_store(j, slice(T + HT, 2 * T), 33)
                for i in range(NZ):
                    scalar.wait_ge(dout[i], 16 * n_store[i])

    return nc


def kernel(x, weight):
    global _last_results
    import ml_dtypes
    from concourse.bass_utils import run_bass_kernel_spmd

    bf16 = ml_dtypes.bfloat16
    x = np.asarray(x, dtype=np.float32)
    weight = np.asarray(weight, dtype=np.float32)

    nc = _build_program()

    in_maps = []
    for core in range(N_CORES):
        sl = slice(core * HC, (core + 1) * HC)
        # [B, S, HC] -> [B, HC, S] -> [ROWS, S] with 3 leading zero columns
        # (the causal padding), row r = b*HC + c
        xs = np.zeros((ROWS, S + 3), bf16)
        xs[:, 3:] = x[:, :, sl].transpose(0, 2, 1).reshape(ROWS, S).astype(bf16)
        ws = weight[sl, :]  # (HC, K)
        w_host = np.zeros((128, NU * K + 1), np.float32)
        wd_host = np.zeros((128, NU * K * 128), bf16)
        idx = np.arange(128)
        for u in range(NU):
            blk = u % (HC // 128)
            wu = ws[blk * 128 : (blk + 1) * 128, :]  # (128, K)
            w_host[:, u * K : (u + 1) * K] = wu
            for i in range(K):
                wd_host[idx, (u * K + i) * 128 + idx] = wu[:, i].astype(bf16)
        in_maps.append({"x": xs, "w": w_host, "wd": wd_host})

    res = run_bass_kernel_spmd(nc, in_maps, list(range(N_CORES)))
    _last_results = res

    out = np.empty((B, S, H), np.float32)
    for core in range(N_CORES):
        sl = slice(core * HC, (core + 1) * HC)
        yc = res.results[core]["y"].astype(np.float32).reshape(B, HC, S)
        out[:, :, sl] = yc.transpose(0, 2, 1)
    return out


# revision 28
# speedup vs baseline: 1.0253x; 1.0020x over previous
"""Causal depthwise conv1d (K=4) + SiLU, sharded over 8 NeuronCores.

Full shapes: x [4, 8192, 2048] f32, weight [2048, 4] f32 -> y [4, 8192, 2048] f32.

Strategy: tensor-parallel over the hidden/channel dim (fully channel
independent, no halo exchange). Each core gets 256 channels, reorganized
host-side to channel-major [B*256, 3+S] (3 leading zero columns provide the
causal padding) so the conv runs along the free dim with channels on SBUF
partitions.

The kernel is HBM-bandwidth bound, so all HBM traffic is bf16: x is converted
host-side (RNE via ml_dtypes), y is written bf16 and upconverted host-side.
That halves traffic vs fp32 (rel-err budget 2e-2 >> bf16's ~2e-3).

Compute split so every engine stays under the ~2.9us/tile bf16 DMA budget:
taps 0-2 run on the TensorEngine as bf16 diagonal-matrix matmuls accumulating
in PSUM (the 32 [128,128] diag matrices are built host-side and DMA'd once);
tap 3 is folded by the DVE scalar_tensor_tensor (z = x3*w3 + psum -> sbuf
bf16); ACT applies SiLU (z -> bf16 out) and triggers the output DMA on its
own HWDGE ring, with inputs streaming on SP's ring.

Ramp shaping (the steady state already runs at the HBM roofline): the diag
DMA is split so unit 0's block lands first and tile 0's input lands in two
halves, pulling the first matmul ~5us earlier; PE/DVE work half-tiles
(per-half sem incs) and the last tile's SiLU + store are split in half, so
the end-of-kernel dependency chain drains ~6us faster.

ACT (silu + store trigger + sem waits) was measured as the steady-state
pacer at ~2.9us/tile, just above the 2.86us DMA floor, and it accumulated a
2-tile backlog by kernel end. So tiles are silued and stored in PAIRS
(adjacent tiles are contiguous in y_d): one 4096-wide ACTIVATE (saves the
352-cycle per-op ramp) and one 1MiB store trigger per two tiles puts ACT at
~2.3us/tile. The last pair keeps the v3 tail exemption (tile 30 single,
tile 31 in halves).

Raw bass (no Tile framework): the installed walrus codegen only accepts one
sync wait per compute instruction, so all synchronization is explicit wait_ge
sequencer instructions. Per-buffer-slot DMA semaphores keep concurrent DMA
completion increments unambiguous. Sem increments fire at instruction
completion, but the sequencer runs ahead, so consumers of an engine's result
always gate on that completion increment (including same-engine self-waits
before DMA triggers).
"""

import contextlib

import numpy as np

B, S, H, K = 4, 8192, 2048, 4
N_CORES = 8
HC = H // N_CORES          # 256 channels per core
ROWS = B * HC              # 1024 rows per core, row r = b*HC + c
NU = ROWS // 128           # 8 partition units
T = 2048                   # token tile
NT = S // T
NTILES = NU * NT           # 32
NB = 10                    # x-tile buffers (deep prefetch rides HBM jitter)
NZ = 4                     # z/y pair buffers (pair-silu blocks need slack)
NC_CHUNK = 512             # one PSUM bank of fp32
NCHUNKS = T // NC_CHUNK
PE_TAPS = 3                # taps on the TensorEngine; tap 3 folds in on DVE
HT = T // 2                # half-tile (sem granularity for PE/DVE)
X0SPLIT = HT + 4           # tile-0 first-half DMA columns (covers chunks 0-1)

_last_results = None       # test harness introspection (exec_time_ns etc.)
_ACT_FUNC = "Silu"         # sim override hook (CoreSim lacks Silu)


def _build_program():
    from concourse import bass, mybir

    f32 = mybir.dt.float32
    bf16 = mybir.dt.bfloat16
    AF = mybir.ActivationFunctionType
    ALU = mybir.AluOpType

    nc = bass.Bass()
    # x arrives with 3 leading zero columns (causal padding): [ROWS, 3+S]
    x_d = nc.declare_dram_parameter("x", [ROWS, S + 3], bf16, isOutput=False)
    # 32 host-built [128,128] diag(w) blocks, unit-major then tap
    wd_d = nc.declare_dram_parameter("wd", [128, NU * K * 128], bf16,
                                     isOutput=False)
    # raw weights (fp32): stt scalar columns + a zeros column (Silu bias)
    w_d = nc.declare_dram_parameter("w", [128, NU * K + 1], f32, isOutput=False)
    y_d = nc.declare_dram_parameter("y", [ROWS, S], bf16, isOutput=True)

    with contextlib.ExitStack() as st:
        wt = st.enter_context(nc.sbuf_tensor("wt", [128, NU * K + 1], f32))
        wdg = st.enter_context(nc.sbuf_tensor("wdg", [128, NU * K * 128], bf16))
        xts = [
            st.enter_context(nc.sbuf_tensor(f"xt{i}", [128, T + 3], bf16))
            for i in range(NB)
        ]
        # pair-sized (2 tiles wide) pre-silu and post-silu buffers
        zps = [
            st.enter_context(nc.sbuf_tensor(f"zp{i}", [128, 2 * T], bf16))
            for i in range(NZ)
        ]
        ytp = [
            st.enter_context(nc.sbuf_tensor(f"yp{i}", [128, 2 * T], bf16))
            for i in range(NZ)
        ]
        pss = [
            st.enter_context(nc.psum_tensor(f"ps{i}", [128, T], f32))
            for i in range(2)
        ]
        zb = wt[:, NU * K : NU * K + 1]           # zeros column (Silu bias)

        def wdiag(k, i):
            u = k // NT
            c0 = (u * K + i) * 128
            return wdg[:, c0 : c0 + 128]

        def w3col(k):
            u = k // NT
            return wt[:, u * K + 3 : u * K + 4]

        def x_rows(k):
            r0 = (k // NT) * 128
            return r0, r0 + 128

        # cumulative din counts: tile 0 arrives as two half DMAs
        din_need = []          # din count PE needs before tile k (full tile)
        din_tot = [0] * NB
        for k in range(NTILES):
            din_tot[k % NB] += 32 if k == 0 else 16
            din_need.append(din_tot[k % NB])

        with (
            nc.Block() as block,
            nc.semaphore("wsem") as wsem,
            nc.semaphore("dsem0") as dsem0,
            nc.semaphore("dsem1") as dsem1,
            nc.semaphore("dvez") as dvez,
            nc.semaphore("act") as act,
            nc.semaphore("pe") as pe,
            contextlib.ExitStack() as sems,
        ):
            din = [
                sems.enter_context(nc.semaphore(f"din{i}")) for i in range(NB)
            ]
            dout = [
                sems.enter_context(nc.semaphore(f"dout{i}")) for i in range(NZ)
            ]

            @block.sync
            def _(sync):
                for k in range(NTILES):
                    r0, r1 = x_rows(k)
                    t0 = (k % NT) * T
                    if k >= NB:
                        # xt slot free once DVE folded tap 3 of tile k-NB
                        sync.wait_ge(dvez, k - NB + 1)
                    # padded coords: window [t0-3, t0+T) = x_d cols [t0, t0+T+3)
                    if k == 0:
                        sync.dma_start(
                            out=xts[0][:, :X0SPLIT],
                            in_=x_d[r0:r1, :X0SPLIT],
                        ).then_inc(din[0], 16)
                        sync.dma_start(
                            out=xts[0][:, X0SPLIT : T + 3],
                            in_=x_d[r0:r1, X0SPLIT : T + 3],
                        ).then_inc(din[0], 16)
                    else:
                        sync.dma_start(
                            out=xts[k % NB][:, :],
                            in_=x_d[r0:r1, t0 : t0 + T + 3],
                        ).then_inc(din[k % NB], 16)

            @block.tensor
            def _(tensor):
                tensor.wait_ge(dsem0, 16)
                for k in range(NTILES):
                    if k == 4:
                        tensor.wait_ge(dsem1, 16)
                    if k >= 2:
                        # psum buffer free once DVE consumed tile k-2
                        tensor.wait_ge(dvez, k - 1)
                    ps = pss[k % 2]
                    xt = xts[k % NB]
                    # last two tiles: all 4 taps on PE (ACT silus straight
                    # from PSUM) so no DVE fold sits in the drain chain
                    taps = K if k >= NTILES - 2 else PE_TAPS
                    for h in range(2):
                        if k == 0:
                            tensor.wait_ge(din[0], 16 * (h + 1))
                        elif h == 0:
                            tensor.wait_ge(din[k % NB], din_need[k])
                        for c in range(2 * h, 2 * h + 2):
                            c0 = c * NC_CHUNK
                            for i in range(taps):
                                mm = tensor.matmul(
                                    ps[:, c0 : c0 + NC_CHUNK],
                                    wdiag(k, i),
                                    xt[:, c0 + i : c0 + i + NC_CHUNK],
                                    start=(i == 0),
                                    stop=(i == taps - 1),
                                    skip_group_check=True,
                                )
                        if h == 1 or k == NTILES - 1:
                            # one inc per steady tile; halves for the last
                            # (pe counts: k+1 for k<=30, then 32, 33)
                            mm.then_inc(pe)

            @block.vector
            def _(vector):
                vector.wait_ge(wsem, 16)
                # last two tiles have no DVE fold (4-tap PE + psum silu)
                for k in range(NTILES - 2):
                    j, e = k // 2, k % 2
                    if e == 0 and j >= NZ:
                        # z pair slot free once ACT silued pair j-NZ
                        vector.wait_ge(act, j - NZ + 1)
                    vector.wait_ge(pe, k + 1)
                    # z = x3 * w3 + psum  (tap 3 fold)
                    vector.scalar_tensor_tensor(
                        out=zps[j % NZ][:, e * T : e * T + T],
                        in0=xts[k % NB][:, 3 : 3 + T],
                        scalar=w3col(k),
                        in1=pss[k % 2][:, :],
                        op0=ALU.mult,
                        op1=ALU.add,
                    ).then_inc(dvez)

            @block.scalar
            def _(scalar):
                scalar.dma_start(
                    out=wdg[:, : K * 128], in_=wd_d[:, : K * 128]
                ).then_inc(dsem0, 16)
                scalar.dma_start(out=wt[:, :], in_=w_d[:, :]).then_inc(wsem, 16)
                scalar.dma_start(
                    out=wdg[:, K * 128 :], in_=wd_d[:, K * 128 :]
                ).then_inc(dsem1, 16)
                func = getattr(AF, _ACT_FUNC)
                bias = 0.0 if func == AF.Copy else zb
                NPAIR = NTILES // 2
                n_act = 0
                n_store = [0] * NZ

                def silu_store(j, sl, wait_sem, wait_val, src):
                    # silu src -> ytp[j%NZ][:, sl], then store that slice
                    nonlocal n_act
                    k0 = 2 * j
                    r0, r1 = x_rows(k0)
                    t0 = (k0 % NT) * T
                    scalar.wait_ge(wait_sem, wait_val)
                    scalar.activation(
                        out=ytp[j % NZ][:, sl], in_=src,
                        func=func, bias=bias, scale=1.0,
                    ).then_inc(act)
                    n_act += 1
                    # the DMA trigger races ahead of the still-streaming
                    # activation write; self-wait on its completion inc
                    scalar.wait_ge(act, n_act)
                    scalar.dma_start(
                        out=y_d[r0:r1, t0 + sl.start : t0 + sl.stop],
                        in_=ytp[j % NZ][:, sl],
                    ).then_inc(dout[j % NZ], 16)
                    n_store[j % NZ] += 1

                for j in range(NPAIR):
                    z = zps[j % NZ]
                    if j >= NZ:
                        # yt pair slot's previous store must be done
                        scalar.wait_ge(dout[j % NZ], 16 * (j // NZ))
                    if j < NPAIR - 2:
                        silu_store(j, slice(0, 2 * T), dvez, 2 * j + 2,
                                   z[:, : 2 * T])
                    elif j == NPAIR - 2:
                        # tail: single tiles so ACT never backlogs behind a
                        # pair-sized silu while the final tiles drain
                        silu_store(j, slice(0, T), dvez, 29, z[:, :T])
                        silu_store(j, slice(T, 2 * T), dvez, 30, z[:, T:])
                    else:
                        # final pair is 4-tap on PE: silu straight from PSUM
                        # (pe counts: k+1 for k<=30, then halves 32, 33)
                        silu_store(j, slice(0, T), pe, 31, pss[0][:, :])
                        silu_store(j, slice(T, T + HT), pe, 32,
                                   pss[1][:, :HT])
                        silu_store(j, slice(T + HT, 2 * T), pe, 33,
                                   pss[1][:, HT:T])
                for i in range(NZ):
                    scalar.wait_ge(dout[i], 16 * n_store[i])

    return nc


def kernel(x, weight):
    global _last_results
    import ml_dtypes
    from concourse.bass_utils import run_bass_kernel_spmd

    bf16 = ml_dtypes.bfloat16
    x = np.asarray(x, dtype=np.float32)
    weight = np.asarray(weight, dtype=np.float32)

    nc = _build_program()

    in_maps = []
    for core in range(N_CORES):
        sl = slice(core * HC, (core + 1) * HC)
        # [B, S, HC] -> [B, HC, S] -> [ROWS, S] with 3 leading zero columns
        # (the causal padding), row r = b*HC + c
        xs = np.zeros((ROWS, S + 3), bf16)
        xs[:, 3:] = x[:, :, sl].transpose(0, 2, 1).reshape(ROWS, S).astype(bf16)
        ws = weight[sl, :]  # (HC, K)
        w_host = np.zeros((128, NU * K + 1), np.float32)
        wd_host = np.zeros((128, NU * K * 128), bf16)
        idx = np.arange(128)
        for u in range(NU):
            blk = u % (HC // 128)
            wu = ws[blk * 128 : (blk + 1) * 128, :]  # (128, K)
            w_host[:, u * K : (u + 1) * K] = wu
            for i in range(K):
                wd_host[idx, (u * K + i) * 128 + idx] = wu[:, i].astype(bf16)
        in_maps.append({"x": xs, "w": w_host, "wd": wd_host})

    res = run_bass_kernel_spmd(nc, in_maps, list(range(N_CORES)))
    _last_results = res

    out = np.empty((B, S, H), np.float32)
    for core in range(N_CORES):
        sl = slice(core * HC, (core + 1) * HC)
        yc = res.results[core]["y"].astype(np.float32).reshape(B, HC, S)
        out[:, :, sl] = yc.transpose(0, 2, 1)
    return out
